# revision 3
# baseline (speedup 1.0000x reference)
"""BotRGCN (4 shared RGCN layers) on 8 TRN2 NeuronCores via Bass/Tile.

Strategy (sharding_hint): nodes sharded across 8 cores (6250 each, padded to
6656 = 13*512); edges partitioned by destination core and sorted by
(dst_local, rel) segment; per layer an AllGather replicates the row-major x
table (fp16) to every core's DRAM, then each core dma_gathers its edges'
source rows and computes segment means via PE matmuls against 0/1 membership
matrices (fp8, resident in SBUF; the per-segment 1/cnt is applied as an
exact per-slot scale on the gathered tiles). Per-relation RGCN weights +
root term are dense PE matmuls; small weights replicated.

v2 over baseline:
  - per-core degree-descending node permutation aligns edge counts across
    cores, tightening the shared span packing (fewer gather slots/tiles);
  - membership matrices are 0/1 fp8 and SBUF-resident (loaded once) instead
    of 17MB/layer of fp16 streamed from HBM; 1/cnt moves to a DVE
    per-partition scale of the gathered tiles (exact);
  - next layer's row-major table is built chunk-by-chunk during the current
    layer and each AllGather half fires as soon as its rows are ready.

Self-contained: hardcodes all shapes from the problem spec.
"""
import os
import time

import numpy as np
import ml_dtypes

import concourse.bacc as bacc
import concourse.bass as bass
import concourse.mybir as mybir
import concourse.tile as tile
from concourse.bass_utils import run_bass_kernel_spmd
from concourse.masks import make_identity

# ---------------- problem constants (hardcoded from spec) ----------------
NCORES = 8
N = 50000
E = 800000
R = 5
D = 128
FIN = 768 + 768 + 6 + 11          # 1553 concat input features
FINP = 13 * 128                   # padded to 1664
NLOC = N // NCORES                # 6250
CHUNK = 512                       # nodes per chunk
NCHUNK = 13
NPAD = NCHUNK * CHUNK             # 6656 padded nodes/core
NTAB = NCORES * NPAD              # 53248 table rows
BANK = 512                        # segment columns per PSUM bank
BANKS_PER_CHUNK = CHUNK * R // BANK   # 5
NBANK = NCHUNK * BANKS_PER_CHUNK  # 65
NSEG = NPAD * R                   # 33280 dense segment grid per core
HALFROW = NPAD // 2               # 3328: rows per half-table shard
NTABH = NCORES * HALFROW          # 26624 rows per half table (< 32768)
SLOTS = 128                       # edge slots per tile
SUBT = 8                          # gather tiles per dma_gather call
NQ = 2                            # SWDGE queues
NLAYER = int(os.environ.get("KB_LAYERS", "4"))
SKIP = set(os.environ.get("KB_SKIP", "").split(","))

F16 = mybir.dt.float16
F32 = mybir.dt.float32
F8 = mybir.dt.float8e4
I16 = mybir.dt.int16

_CACHE = {}


# ---------------- host-side graph preprocessing ----------------
def _plan_graph(edge_index, edge_type):
    """Build per-core tile structure. Span layout is shared by all cores
    (SPMD: one program), per-core data (idx, M, inv) differs.

    Nodes within each core are permuted by descending total degree so the
    per-column edge counts correlate across cores — the greedy shared span
    packing then wastes fewer slots on the max-core constraint."""
    src = np.asarray(edge_index[0], dtype=np.int64)
    dst = np.asarray(edge_index[1], dtype=np.int64)
    et = np.asarray(edge_type, dtype=np.int64)

    # per-core degree-descending permutation: order[k][j] = original local
    # id of the node placed at permuted position j on core k.
    dst_core_all = dst // NLOC
    dst_loc_all = dst % NLOC
    order = np.zeros((NCORES, NLOC), dtype=np.int64)
    pos_of = np.zeros((NCORES, NLOC), dtype=np.int64)
    for k in range(NCORES):
        degk = np.bincount(dst_loc_all[dst_core_all == k], minlength=NLOC)
        o = np.argsort(-degk, kind="stable")
        order[k] = o
        pos_of[k, o] = np.arange(NLOC)

    core = dst_core_all
    col = pos_of[core, dst_loc_all] * R + et          # permuted (node, rel)
    src_core = src // NLOC
    src_pos = pos_of[src_core, src % NLOC]            # permuted src position
    stream = (src_pos >= HALFROW).astype(np.int64)    # src half
    src_pad = src_core * HALFROW + (src_pos - stream * HALFROW)

    # per (core, stream): edges sorted by col
    edges = {}
    counts = np.zeros((NCORES, 2, NSEG), dtype=np.int64)
    for k in range(NCORES):
        for s in range(2):
            m = (core == k) & (stream == s)
            c = col[m]
            o = np.argsort(c, kind="stable")
            edges[(k, s)] = (c[o], src_pad[m][o])
            np.add.at(counts[k, s], c[o], 1)

    cnt_total = counts.sum(axis=1)                    # [NCORES, NSEG]
    invc = 1.0 / np.maximum(cnt_total, 1.0)           # per core

    # static spans per (stream, bank): greedy, max-over-cores count <= SLOTS
    spans = {0: [], 1: []}                            # spans[s][b] = [widths]
    for s in range(2):
        for b in range(NBANK):
            base = b * BANK
            cc = counts[:, s, base:base + BANK]       # [NCORES, BANK]
            assert cc.max(initial=0) <= SLOTS, "single segment exceeds tile"
            widths = []
            run = np.zeros(NCORES, dtype=np.int64)
            w = 0
            for j in range(BANK):
                if (run + cc[:, j]).max() > SLOTS:
                    widths.append(w)
                    run[:] = 0
                    w = 0
                run += cc[:, j]
                w += 1
            widths.append(w)
            spans[s].append(widths)

    ntiles = {s: [len(spans[s][b]) for b in range(NBANK)] for s in range(2)}
    call_tiles = {s: [sum(ntiles[s][c * BANKS_PER_CHUNK + b]
                          for b in range(BANKS_PER_CHUNK))
                      for c in range(NCHUNK)] for s in range(2)}
    tot_tiles = {s: sum(ntiles[s]) for s in range(2)}
    tt_all = tot_tiles[0] + tot_tiles[1]

    # per-core data: gather idx (wrapped int16), 0/1 M matrices (fp8),
    # per-slot 1/cnt scales
    gidx = {s: np.zeros((NCORES, 128, tot_tiles[s] * SLOTS // 16), np.int16)
            for s in range(2)}
    mmat = np.zeros((NCORES, 128, 2 * NBANK * BANK), ml_dtypes.float8_e4m3)
    invslot = np.ones((NCORES, 128, tt_all), np.float32)
    for k in range(NCORES):
        for s in range(2):
            cols_e, srcp_e = edges[(k, s)]
            flat_idx = np.zeros(tot_tiles[s] * SLOTS, np.int16)
            tglob = 0
            for b in range(NBANK):
                base = b * BANK
                lo = 0
                for w in spans[s][b]:
                    e0 = np.searchsorted(cols_e, base + lo)
                    e1 = np.searchsorted(cols_e, base + lo + w)
                    nslot = e1 - e0
                    assert nslot <= SLOTS
                    flat_idx[tglob * SLOTS:tglob * SLOTS + nslot] = \
                        srcp_e[e0:e1]
                    mcol = (s * NBANK + b) * BANK + (cols_e[e0:e1] - base)
                    mmat[k, np.arange(nslot), mcol] = 1.0
                    invslot[k, :nslot, s * tot_tiles[0] + tglob] = \
                        invc[k][cols_e[e0:e1]].astype(np.float32)
                    lo += w
                    tglob += 1
            # wrap: element i -> [i%16, i//16], replicated across 8 groups
            wr = flat_idx.reshape(-1, 16).T            # [16, ntot*8]
            gidx[s][k] = np.tile(wr, (8, 1))
    return dict(spans=spans, ntiles=ntiles, call_tiles=call_tiles,
                tot_tiles=tot_tiles, gidx=gidx, mmat=mmat, invslot=invslot,
                order=order)


# ---------------- device program ----------------
def _build_nc(plan):
    nc = bacc.Bacc("TRN2", target_bir_lowering=False, debug=False,
                   num_devices=NCORES, num_swdge_queues=NQ,
                   dynamic_dma_scratch_size=32768)
    spans, ntiles = plan["spans"], plan["ntiles"]
    call_tiles, tot_tiles = plan["call_tiles"], plan["tot_tiles"]
    tt_all = tot_tiles[0] + tot_tiles[1]

    # inputs (per core)
    featT = nc.dram_tensor("featT", [FINP, NPAD], F16, kind="ExternalInput")
    w_all = nc.dram_tensor("w_all", [128, 13 * 128], F16, kind="ExternalInput")
    b_x0 = nc.dram_tensor("b_x0", [128, 1], F32, kind="ExternalInput")
    w_in = nc.dram_tensor("w_in", [128, 128], F16, kind="ExternalInput")
    b_in = nc.dram_tensor("b_in", [128, 1], F32, kind="ExternalInput")
    relw = nc.dram_tensor("relw", [128, R * 128], F16, kind="ExternalInput")
    rootw = nc.dram_tensor("rootw", [128, 128], F16, kind="ExternalInput")
    rgcn_b = nc.dram_tensor("rgcn_b", [128, 1], F32, kind="ExternalInput")
    wo1 = nc.dram_tensor("wo1", [128, 128], F16, kind="ExternalInput")
    b_o1 = nc.dram_tensor("b_o1", [128, 1], F32, kind="ExternalInput")
    wo2 = nc.dram_tensor("wo2", [128, 2], F16, kind="ExternalInput")
    b_o2 = nc.dram_tensor("b_o2", [2, 1], F32, kind="ExternalInput")
    gidxA = nc.dram_tensor("gidxA", [128, tot_tiles[0] * 8], I16,
                           kind="ExternalInput")
    gidxB = nc.dram_tensor("gidxB", [128, tot_tiles[1] * 8], I16,
                           kind="ExternalInput")
    mmat = nc.dram_tensor("mmat", [128, 2 * NBANK * BANK], F8,
                          kind="ExternalInput")
    invslot = nc.dram_tensor("invslot", [128, tt_all], F32,
                             kind="ExternalInput")
    outT = nc.dram_tensor("outT", [2, NPAD], F32, kind="ExternalOutput")

    with tile.TileContext(nc) as tc:
        with (
            tc.tile_pool(name="const", bufs=1) as constp,
            tc.tile_pool(name="xt", bufs=2) as xtp,
            tc.tile_pool(name="feat", bufs=3) as featp,
            tc.tile_pool(name="gb", bufs=4) as gbp,
            tc.tile_pool(name="stile", bufs=2) as stp,
            tc.tile_pool(name="small", bufs=3) as smallp,
            tc.tile_pool(name="pbank", bufs=3, space="PSUM") as pbank,
            tc.tile_pool(name="pbig", bufs=2, space="PSUM") as pbig,
            tc.tile_pool(name="ptp", bufs=2, space="PSUM") as ptpp,
            tc.tile_pool(name="dram", bufs=1, space="DRAM") as dramp,
            tc.tile_pool(name="shared", bufs=1, space="DRAM") as sharedp,
        ):
            # ---- resident constants ----
            def load_const(t, shape, dt):
                s = constp.tile(shape, dt, tag=t.name)
                nc.sync.dma_start(s[:], t[:])
                return s
            w_all_s = load_const(w_all, [128, 13 * 128], F16)
            b_x0_s = load_const(b_x0, [128, 1], F32)
            w_in_s = load_const(w_in, [128, 128], F16)
            b_in_s = load_const(b_in, [128, 1], F32)
            relw_s = load_const(relw, [128, R * 128], F16)
            rootw_s = load_const(rootw, [128, 128], F16)
            rgcn_b_s = load_const(rgcn_b, [128, 1], F32)
            wo1_s = load_const(wo1, [128, 128], F16)
            b_o1_s = load_const(b_o1, [128, 1], F32)
            wo2_s = load_const(wo2, [128, 2], F16)
            b_o2_s = load_const(b_o2, [2, 1], F32)
            gidx_s = [load_const(gidxA, [128, tot_tiles[0] * 8], I16),
                      load_const(gidxB, [128, tot_tiles[1] * 8], I16)]
            mmat_s = load_const(mmat, [128, 2 * NBANK * BANK], F8)
            invslot_s = load_const(invslot, [128, tt_all], F32)
            ident = constp.tile([128, 128], F16, tag="ident")
            make_identity(nc, ident[:])

            # table build: transpose the 4 blocks of chunk c of `xsrc` into
            # `tstage`; on the half boundaries ship + AllGather into `tables`
            def build_tables_postchunk(layer, c, xsrc, tstage, tables):
                for j in range(4 * c, 4 * c + 4):
                    pt = ptpp.tile([128, 128], F16, space="PSUM", tag="ptp")
                    nc.tensor.transpose(pt[:], xsrc[:, j * 128:(j + 1) * 128],
                                        ident[:])
                    nc.vector.tensor_copy(tstage[:, j * 128:(j + 1) * 128],
                                          pt[:])
                for h in range(2):
                    if c != (6 if h == 0 else 12):
                        continue
                    tb = sharedp.tile([NTABH, D], F16, addr_space="Shared",
                                      tag=f"table{layer}_{h}")
                    tsh = dramp.tile([HALFROW, D], F16, tag=f"tsh{layer}_{h}")
                    nc.sync.dma_start(
                        tsh[:].rearrange("(j p) d -> p j d", p=128),
                        tstage[:, h * HALFROW:(h + 1) * HALFROW].rearrange(
                            "p (j d) -> p j d", d=D))
                    if "coll" not in SKIP:
                        nc.gpsimd.collective_compute(
                            "AllGather", mybir.AluOpType.bypass,
                            replica_groups=[list(range(NCORES))],
                            ins=[tsh[:].opt()], outs=[tb[:].opt()])
                    else:
                        nc.sync.dma_start(tb[NPAD // 2:NPAD, :], tsh[:])
                    tables[h] = tb

            # ---- input projection -> xT [128, NPAD] fp16 (+ layer-0 table)
            xT = xtp.tile([128, NPAD], F16, tag="xT")
            tstage = xtp.tile([128, NPAD], F16, tag="tstage")
            tables = [None, None]
            for c in range(NCHUNK):
                cs = slice(c * CHUNK, (c + 1) * CHUNK)
                p0 = pbig.tile([128, CHUNK], F32, space="PSUM", tag="pbig")
                for f in range(13):
                    ft = featp.tile([128, CHUNK], F16, tag="feat")
                    nc.sync.dma_start(ft[:], featT[f * 128:(f + 1) * 128, cs])
                    nc.tensor.matmul(p0[:],
                                     lhsT=w_all_s[:, f * 128:(f + 1) * 128],
                                     rhs=ft[:], start=(f == 0), stop=(f == 12))
                x0 = smallp.tile([128, CHUNK], F16, tag="x0")
                nc.scalar.activation(x0[:], p0[:],
                                     mybir.ActivationFunctionType.Lrelu,
                                     bias=b_x0_s[:], scale=1.0, alpha=0.01)
                p1 = pbig.tile([128, CHUNK], F32, space="PSUM", tag="pbig")
                nc.tensor.matmul(p1[:], lhsT=w_in_s[:], rhs=x0[:],
                                 start=True, stop=True)
                nc.scalar.activation(xT[:, cs], p1[:],
                                     mybir.ActivationFunctionType.Lrelu,
                                     bias=b_in_s[:], scale=1.0, alpha=0.01)
                build_tables_postchunk(0, c, xT, tstage, tables)

            # ---- RGCN layers ----
            for layer in range(NLAYER):
                xTn = xtp.tile([128, NPAD], F16, tag="xT")
                if layer + 1 < NLAYER:
                    tstage_n = xtp.tile([128, NPAD], F16, tag="tstage")
                    tables_n = [None, None]
                goffs = {0: 0, 1: 0}      # gather idx cursor per stream
                qsel = 0
                for c in range(NCHUNK):
                    # gather: sub-calls of <= SUBT tiles (descriptor-ring cap)
                    gtiles = {}
                    gscales = {}
                    for s in range(2):
                        tc_s = call_tiles[s][c]
                        view = tables[s][:]
                        subs = []
                        scl = []
                        for t0 in range(0, tc_s, SUBT):
                            nt = min(SUBT, tc_s - t0)
                            gb = gbp.tile([128, SUBT, D], F16, tag=f"gb{s}")
                            ni = nt * SLOTS
                            if "gather" not in SKIP:
                                nc.gpsimd.dma_gather(
                                    gb[:, :nt, :], view, gidx_s[s][
                                        :, goffs[s]:goffs[s] + ni // 16],
                                    ni, ni, D, queue_num=qsel % NQ)
                                qsel += 1
                            else:
                                nc.vector.memset(gb[:, :nt, :], 0.0)
                            # exact per-slot 1/cnt scale (in place, DVE)
                            tbase = s * tot_tiles[0] + goffs[s] // 8
                            for tt in range(nt):
                                nc.vector.tensor_scalar_mul(
                                    gb[:, tt, :], gb[:, tt, :],
                                    invslot_s[:, tbase + tt:tbase + tt + 1])
                            goffs[s] += ni // 16
                            subs.append(gb)
                        gtiles[s] = subs
                    st = stp.tile([128, CHUNK * R], F16, tag="stile")
                    for b in range(BANKS_PER_CHUNK):
                        bg = c * BANKS_PER_CHUNK + b
                        pb = pbank.tile([128, BANK], F32, space="PSUM",
                                        tag="pbank")
                        n_mm = len(spans[0][bg]) + len(spans[1][bg])
                        i_mm = 0
                        for s in range(2):
                            mbase = (s * NBANK + bg) * BANK
                            lo = 0
                            tloc = sum(ntiles[s][c * BANKS_PER_CHUNK + bb]
                                       for bb in range(b))
                            for w in spans[s][bg]:
                                nc.tensor.matmul(
                                    pb[:, lo:lo + w],
                                    lhsT=gtiles[s][tloc // SUBT][
                                        :, tloc % SUBT, :],
                                    rhs=mmat_s[:, mbase + lo:mbase + lo + w],
                                    start=(i_mm == 0),
                                    stop=(i_mm == n_mm - 1))
                                lo += w
                                tloc += 1
                                i_mm += 1
                            assert lo == BANK
                        assert i_mm == n_mm
                        nc.vector.tensor_copy(st[:, b * BANK:(b + 1) * BANK],
                                              pb[:])
                    # phase 2: per-relation + root matmuls
                    cs = slice(c * CHUNK, (c + 1) * CHUNK)
                    po = pbig.tile([128, CHUNK], F32, space="PSUM", tag="pbig")
                    str_ap = st[:].rearrange("p (n r) -> p r n", r=R)
                    for r in range(R):
                        nc.tensor.matmul(po[:],
                                         lhsT=relw_s[:, r * 128:(r + 1) * 128],
                                         rhs=str_ap[:, r, :],
                                         start=(r == 0), stop=False)
                    nc.tensor.matmul(po[:], lhsT=rootw_s[:], rhs=xT[:, cs],
                                     start=False, stop=True)
                    nc.scalar.activation(xTn[:, cs], po[:],
                                         mybir.ActivationFunctionType.Identity,
                                         bias=rgcn_b_s[:], scale=1.0)
                    if layer + 1 < NLAYER:
                        build_tables_postchunk(layer + 1, c, xTn, tstage_n,
                                               tables_n)
                xT = xTn
                if layer + 1 < NLAYER:
                    tstage = tstage_n
                    tables = tables_n

            # ---- output head ----
            for c in range(NCHUNK):
                cs = slice(c * CHUNK, (c + 1) * CHUNK)
                p1 = pbig.tile([128, CHUNK], F32, space="PSUM", tag="pbig")
                nc.tensor.matmul(p1[:], lhsT=wo1_s[:], rhs=xT[:, cs],
                                 start=True, stop=True)
                h = smallp.tile([128, CHUNK], F16, tag="x0")
                nc.scalar.activation(h[:], p1[:],
                                     mybir.ActivationFunctionType.Lrelu,
                                     bias=b_o1_s[:], scale=1.0, alpha=0.01)
                p2 = ptpp.tile([2, CHUNK], F32, space="PSUM", tag="ptp")
                nc.tensor.matmul(p2[:], lhsT=wo2_s[:], rhs=h[:],
                                 start=True, stop=True)
                ot = smallp.tile([2, CHUNK], F32, tag="ot")
                nc.scalar.activation(ot[:], p2[:],
                                     mybir.ActivationFunctionType.Identity,
                                     bias=b_o2_s[:], scale=1.0)
                nc.sync.dma_start(outT[:, cs], ot[:])

    nc.compile()
    return nc


# ---------------- host wrapper ----------------
def _pack_inputs(inputs, plan):
    f16 = np.float16
    des, tweet = inputs["des"], inputs["tweet"]
    num_prop, cat_prop = inputs["num_prop"], inputs["cat_prop"]
    order = plan["order"]

    w_blk = np.zeros((FINP, 128), np.float32)
    w_blk[0:768, 0:32] = inputs["W_des"]
    w_blk[768:1536, 32:64] = inputs["W_tw"]
    w_blk[1536:1542, 64:96] = inputs["W_np"]
    w_blk[1542:1553, 96:128] = inputs["W_cp"]
    w_all = np.concatenate([w_blk[f * 128:(f + 1) * 128, :]
                            for f in range(13)], axis=1).astype(f16)
    b_x0 = np.concatenate([inputs["b_des"], inputs["b_tw"],
                           inputs["b_np"], inputs["b_cp"]]
                          ).astype(np.float32).reshape(128, 1)
    relw = np.concatenate([inputs["rel_w"][r] for r in range(R)],
                          axis=1).astype(f16)

    in_maps = []
    for k in range(NCORES):
        rows = k * NLOC + order[k]                   # permuted global rows
        feat = np.zeros((FINP, NPAD), f16)
        feat[0:768, :NLOC] = des[rows].T
        feat[768:1536, :NLOC] = tweet[rows].T
        feat[1536:1542, :NLOC] = num_prop[rows].T
        feat[1542:1553, :NLOC] = cat_prop[rows].T
        m = {
            "featT": feat,
            "w_all": w_all,
            "b_x0": b_x0,
            "w_in": inputs["W_in"].astype(f16),
            "b_in": inputs["b_in"].astype(np.float32).reshape(128, 1),
            "relw": relw,
            "rootw": inputs["root_w"].astype(f16),
            "rgcn_b": inputs["rgcn_b"].astype(np.float32).reshape(128, 1),
            "wo1": inputs["W_o1"].astype(f16),
            "b_o1": inputs["b_o1"].astype(np.float32).reshape(128, 1),
            "wo2": inputs["W_o2"].astype(f16),
            "b_o2": inputs["b_o2"].astype(np.float32).reshape(2, 1),
            "gidxA": plan["gidx"][0][k],
            "gidxB": plan["gidx"][1][k],
            "mmat": plan["mmat"][k],
            "invslot": plan["invslot"][k],
        }
        in_maps.append(m)
    return in_maps


def _get_compiled(edge_index, edge_type):
    key = hash((np.asarray(edge_index).tobytes(),
                np.asarray(edge_type).tobytes()))
    if key not in _CACHE:
        t0 = time.time()
        plan = _plan_graph(edge_index, edge_type)
        t1 = time.time()
        nc = _build_nc(plan)
        t2 = time.time()
        print(f"[kernel] plan {t1-t0:.0f}s, build+compile {t2-t1:.0f}s",
              flush=True)
        _CACHE[key] = (nc, plan)
    return _CACHE[key]


def kernel(trace=False, **inputs):
    nc, plan = _get_compiled(inputs["edge_index"], inputs["edge_type"])
    in_maps = _pack_inputs(inputs, plan)
    t0 = time.time()
    res = run_bass_kernel_spmd(nc, in_maps, list(range(NCORES)), trace=trace)
    print(f"[kernel] run {time.time()-t0:.0f}s", flush=True)
    out = np.zeros((N, 2), np.float32)
    order = plan["order"]
    for k in range(NCORES):
        out[k * NLOC + order[k]] = res.results[k]["outT"][:, :NLOC].T
    if trace:
        return out, res
    return out


# revision 4
# speedup vs baseline: 1.2517x; 1.2517x over previous
"""BotRGCN (4 shared RGCN layers) on 8 TRN2 NeuronCores via Bass/Tile.

Strategy (sharding_hint): nodes sharded across 8 cores (6250 each, padded to
6656 = 13*512); edges partitioned by destination core and sorted by
(dst_local, rel) segment; per layer an AllGather replicates the row-major x
table (fp16) to every core's DRAM, then each core dma_gathers its edges'
source rows and computes segment means via PE matmuls against 0/1 membership
matrices (fp8, resident in SBUF; the per-segment 1/cnt is applied as an
exact per-slot scale on the gathered tiles). Per-relation RGCN weights +
root term are dense PE matmuls; small weights replicated.

v2 over baseline:
  - per-core degree-descending node permutation aligns edge counts across
    cores, tightening the shared span packing (fewer gather slots/tiles);
  - membership matrices are 0/1 fp8 and SBUF-resident (loaded once) instead
    of 17MB/layer of fp16 streamed from HBM; 1/cnt moves to a DVE
    per-partition scale of the gathered tiles (exact);
  - next layer's row-major table is built chunk-by-chunk during the current
    layer and each AllGather half fires as soon as its rows are ready.

Self-contained: hardcodes all shapes from the problem spec.
"""
import os
import time

import numpy as np
import ml_dtypes

import concourse.bacc as bacc
import concourse.bass as bass
import concourse.mybir as mybir
import concourse.tile as tile
from concourse.bass_utils import run_bass_kernel_spmd
from concourse.masks import make_identity

# ---------------- problem constants (hardcoded from spec) ----------------
NCORES = 8
N = 50000
E = 800000
R = 5
D = 128
FIN = 768 + 768 + 6 + 11          # 1553 concat input features
FINP = 13 * 128                   # padded to 1664
NLOC = N // NCORES                # 6250
CHUNK = 512                       # nodes per chunk
NCHUNK = 13
NPAD = NCHUNK * CHUNK             # 6656 padded nodes/core
NTAB = NCORES * NPAD              # 53248 table rows
BANK = 512                        # segment columns per PSUM bank
BANKS_PER_CHUNK = CHUNK * R // BANK   # 5
NBANK = NCHUNK * BANKS_PER_CHUNK  # 65
NSEG = NPAD * R                   # 33280 dense segment grid per core
HALFROW = NPAD // 2               # 3328: rows per half-table shard
NTABH = NCORES * HALFROW          # 26624 rows per half table (< 32768)
SLOTS = 128                       # edge slots per tile
SUBT = int(os.environ.get("KB_SUBT", "8"))   # gather tiles per call
NQ = int(os.environ.get("KB_NQ", "4"))        # SWDGE queues
SCRATCH = int(os.environ.get("KB_SCRATCH", "32768"))
NLAYER = int(os.environ.get("KB_LAYERS", "4"))
SKIP = set(os.environ.get("KB_SKIP", "").split(","))

F16 = mybir.dt.float16
F32 = mybir.dt.float32
F8 = mybir.dt.float8e4
I16 = mybir.dt.int16

_CACHE = {}


# ---------------- host-side graph preprocessing ----------------
def _plan_graph(edge_index, edge_type):
    """Build per-core tile structure. Span layout is shared by all cores
    (SPMD: one program), per-core data (idx, M, inv) differs.

    Nodes within each core are permuted by descending total degree so the
    per-column edge counts correlate across cores — the greedy shared span
    packing then wastes fewer slots on the max-core constraint."""
    src = np.asarray(edge_index[0], dtype=np.int64)
    dst = np.asarray(edge_index[1], dtype=np.int64)
    et = np.asarray(edge_type, dtype=np.int64)

    # per-core degree-descending permutation: order[k][j] = original local
    # id of the node placed at permuted position j on core k.
    dst_core_all = dst // NLOC
    dst_loc_all = dst % NLOC
    order = np.zeros((NCORES, NLOC), dtype=np.int64)
    pos_of = np.zeros((NCORES, NLOC), dtype=np.int64)
    for k in range(NCORES):
        degk = np.bincount(dst_loc_all[dst_core_all == k], minlength=NLOC)
        o = np.argsort(-degk, kind="stable")
        order[k] = o
        pos_of[k, o] = np.arange(NLOC)

    core = dst_core_all
    col = pos_of[core, dst_loc_all] * R + et          # permuted (node, rel)
    src_core = src // NLOC
    src_pos = pos_of[src_core, src % NLOC]            # permuted src position
    stream = (src_pos >= HALFROW).astype(np.int64)    # src half
    src_pad = src_core * HALFROW + (src_pos - stream * HALFROW)

    # per (core, stream): edges sorted by col
    edges = {}
    counts = np.zeros((NCORES, 2, NSEG), dtype=np.int64)
    for k in range(NCORES):
        for s in range(2):
            m = (core == k) & (stream == s)
            c = col[m]
            o = np.argsort(c, kind="stable")
            edges[(k, s)] = (c[o], src_pad[m][o])
            np.add.at(counts[k, s], c[o], 1)

    cnt_total = counts.sum(axis=1)                    # [NCORES, NSEG]
    invc = 1.0 / np.maximum(cnt_total, 1.0)           # per core

    # static spans per (stream, bank): greedy, max-over-cores count <= SLOTS
    spans = {0: [], 1: []}                            # spans[s][b] = [widths]
    for s in range(2):
        for b in range(NBANK):
            base = b * BANK
            cc = counts[:, s, base:base + BANK]       # [NCORES, BANK]
            assert cc.max(initial=0) <= SLOTS, "single segment exceeds tile"
            widths = []
            run = np.zeros(NCORES, dtype=np.int64)
            w = 0
            for j in range(BANK):
                if (run + cc[:, j]).max() > SLOTS:
                    widths.append(w)
                    run[:] = 0
                    w = 0
                run += cc[:, j]
                w += 1
            widths.append(w)
            spans[s].append(widths)

    ntiles = {s: [len(spans[s][b]) for b in range(NBANK)] for s in range(2)}
    call_tiles = {s: [sum(ntiles[s][c * BANKS_PER_CHUNK + b]
                          for b in range(BANKS_PER_CHUNK))
                      for c in range(NCHUNK)] for s in range(2)}
    tot_tiles = {s: sum(ntiles[s]) for s in range(2)}
    tt_all = tot_tiles[0] + tot_tiles[1]

    # per-core data: gather idx (wrapped int16), 0/1 M matrices (fp8),
    # per-slot 1/cnt scales
    gidx = {s: np.zeros((NCORES, 128, tot_tiles[s] * SLOTS // 16), np.int16)
            for s in range(2)}
    mmat = np.zeros((NCORES, 128, 2 * NBANK * BANK), np.float16)
    for k in range(NCORES):
        for s in range(2):
            cols_e, srcp_e = edges[(k, s)]
            flat_idx = np.zeros(tot_tiles[s] * SLOTS, np.int16)
            tglob = 0
            for b in range(NBANK):
                base = b * BANK
                lo = 0
                for w in spans[s][b]:
                    e0 = np.searchsorted(cols_e, base + lo)
                    e1 = np.searchsorted(cols_e, base + lo + w)
                    nslot = e1 - e0
                    assert nslot <= SLOTS
                    flat_idx[tglob * SLOTS:tglob * SLOTS + nslot] = \
                        srcp_e[e0:e1]
                    mcol = (s * NBANK + b) * BANK + (cols_e[e0:e1] - base)
                    mmat[k, np.arange(nslot), mcol] = \
                        invc[k][cols_e[e0:e1]].astype(np.float16)
                    lo += w
                    tglob += 1
            # wrap: element i -> [i%16, i//16], replicated across 8 groups
            wr = flat_idx.reshape(-1, 16).T            # [16, ntot*8]
            gidx[s][k] = np.tile(wr, (8, 1))
    return dict(spans=spans, ntiles=ntiles, call_tiles=call_tiles,
                tot_tiles=tot_tiles, gidx=gidx, mmat=mmat, order=order)


# ---------------- device program ----------------
def _build_nc(plan):
    nc = bacc.Bacc("TRN2", target_bir_lowering=False, debug=False,
                   num_devices=NCORES, num_swdge_queues=NQ,
                   dynamic_dma_scratch_size=SCRATCH)
    spans, ntiles = plan["spans"], plan["ntiles"]
    call_tiles, tot_tiles = plan["call_tiles"], plan["tot_tiles"]
    tt_all = tot_tiles[0] + tot_tiles[1]

    # inputs (per core)
    featT = nc.dram_tensor("featT", [FINP, NPAD], F16, kind="ExternalInput")
    w_all = nc.dram_tensor("w_all", [128, 13 * 128], F16, kind="ExternalInput")
    b_x0 = nc.dram_tensor("b_x0", [128, 1], F32, kind="ExternalInput")
    w_in = nc.dram_tensor("w_in", [128, 128], F16, kind="ExternalInput")
    b_in = nc.dram_tensor("b_in", [128, 1], F32, kind="ExternalInput")
    relw = nc.dram_tensor("relw", [128, R * 128], F16, kind="ExternalInput")
    rootw = nc.dram_tensor("rootw", [128, 128], F16, kind="ExternalInput")
    rgcn_b = nc.dram_tensor("rgcn_b", [128, 1], F32, kind="ExternalInput")
    wo1 = nc.dram_tensor("wo1", [128, 128], F16, kind="ExternalInput")
    b_o1 = nc.dram_tensor("b_o1", [128, 1], F32, kind="ExternalInput")
    wo2 = nc.dram_tensor("wo2", [128, 2], F16, kind="ExternalInput")
    b_o2 = nc.dram_tensor("b_o2", [2, 1], F32, kind="ExternalInput")
    gidxA = nc.dram_tensor("gidxA", [128, tot_tiles[0] * 8], I16,
                           kind="ExternalInput")
    gidxB = nc.dram_tensor("gidxB", [128, tot_tiles[1] * 8], I16,
                           kind="ExternalInput")
    mmat = nc.dram_tensor("mmat", [128, 2 * NBANK * BANK], F16,
                          kind="ExternalInput")
    outT = nc.dram_tensor("outT", [2, NPAD], F32, kind="ExternalOutput")

    with tile.TileContext(nc) as tc:
        with (
            tc.tile_pool(name="const", bufs=1) as constp,
            tc.tile_pool(name="xt", bufs=2) as xtp,
            tc.tile_pool(name="feat", bufs=3) as featp,
            tc.tile_pool(name="gb", bufs=4) as gbp,
            tc.tile_pool(name="msb", bufs=3) as msbp,
            tc.tile_pool(name="stile", bufs=2) as stp,
            tc.tile_pool(name="small", bufs=3) as smallp,
            tc.tile_pool(name="pbank", bufs=3, space="PSUM") as pbank,
            tc.tile_pool(name="pbig", bufs=2, space="PSUM") as pbig,
            tc.tile_pool(name="ptp", bufs=2, space="PSUM") as ptpp,
            tc.tile_pool(name="dram", bufs=1, space="DRAM") as dramp,
            tc.tile_pool(name="shared", bufs=1, space="DRAM") as sharedp,
        ):
            # ---- resident constants ----
            def load_const(t, shape, dt):
                s = constp.tile(shape, dt, tag=t.name)
                nc.sync.dma_start(s[:], t[:])
                return s
            w_all_s = load_const(w_all, [128, 13 * 128], F16)
            b_x0_s = load_const(b_x0, [128, 1], F32)
            w_in_s = load_const(w_in, [128, 128], F16)
            b_in_s = load_const(b_in, [128, 1], F32)
            relw_s = load_const(relw, [128, R * 128], F16)
            rootw_s = load_const(rootw, [128, 128], F16)
            rgcn_b_s = load_const(rgcn_b, [128, 1], F32)
            wo1_s = load_const(wo1, [128, 128], F16)
            b_o1_s = load_const(b_o1, [128, 1], F32)
            wo2_s = load_const(wo2, [128, 2], F16)
            b_o2_s = load_const(b_o2, [2, 1], F32)
            gidx_s = [load_const(gidxA, [128, tot_tiles[0] * 8], I16),
                      load_const(gidxB, [128, tot_tiles[1] * 8], I16)]
            ident = constp.tile([128, 128], F16, tag="ident")
            make_identity(nc, ident[:])

            # table build: transpose the 4 blocks of chunk c of `xsrc` into
            # `tstage`; on the half boundaries ship + AllGather into `tables`
            def build_tables_postchunk(layer, c, xsrc, tstage, tables):
                for j in range(4 * c, 4 * c + 4):
                    pt = ptpp.tile([128, 128], F16, space="PSUM", tag="ptp")
                    nc.tensor.transpose(pt[:], xsrc[:, j * 128:(j + 1) * 128],
                                        ident[:])
                    nc.vector.tensor_copy(tstage[:, j * 128:(j + 1) * 128],
                                          pt[:])
                for h in range(2):
                    if c != (6 if h == 0 else 12):
                        continue
                    tb = sharedp.tile([NTABH, D], F16, addr_space="Shared",
                                      tag=f"table{layer}_{h}")
                    tsh = dramp.tile([HALFROW, D], F16, tag=f"tsh{layer}_{h}")
                    nc.sync.dma_start(
                        tsh[:].rearrange("(j p) d -> p j d", p=128),
                        tstage[:, h * HALFROW:(h + 1) * HALFROW].rearrange(
                            "p (j d) -> p j d", d=D))
                    if "coll" not in SKIP:
                        nc.gpsimd.collective_compute(
                            "AllGather", mybir.AluOpType.bypass,
                            replica_groups=[list(range(NCORES))],
                            ins=[tsh[:].opt()], outs=[tb[:].opt()])
                    else:
                        nc.sync.dma_start(tb[NPAD // 2:NPAD, :], tsh[:])
                    tables[h] = tb

            # ---- input projection -> xT [128, NPAD] fp16 (+ layer-0 table)
            xT = xtp.tile([128, NPAD], F16, tag="xT")
            tstage = xtp.tile([128, NPAD], F16, tag="tstage")
            tables = [None, None]
            for c in range(NCHUNK):
                cs = slice(c * CHUNK, (c + 1) * CHUNK)
                p0 = pbig.tile([128, CHUNK], F32, space="PSUM", tag="pbig")
                for f in range(13):
                    ft = featp.tile([128, CHUNK], F16, tag="feat")
                    nc.sync.dma_start(ft[:], featT[f * 128:(f + 1) * 128, cs])
                    nc.tensor.matmul(p0[:],
                                     lhsT=w_all_s[:, f * 128:(f + 1) * 128],
                                     rhs=ft[:], start=(f == 0), stop=(f == 12))
                x0 = smallp.tile([128, CHUNK], F16, tag="x0")
                nc.scalar.activation(x0[:], p0[:],
                                     mybir.ActivationFunctionType.Lrelu,
                                     bias=b_x0_s[:], scale=1.0, alpha=0.01)
                p1 = pbig.tile([128, CHUNK], F32, space="PSUM", tag="pbig")
                nc.tensor.matmul(p1[:], lhsT=w_in_s[:], rhs=x0[:],
                                 start=True, stop=True)
                nc.scalar.activation(xT[:, cs], p1[:],
                                     mybir.ActivationFunctionType.Lrelu,
                                     bias=b_in_s[:], scale=1.0, alpha=0.01)
                build_tables_postchunk(0, c, xT, tstage, tables)

            # ---- RGCN layers ----
            for layer in range(NLAYER):
                xTn = xtp.tile([128, NPAD], F16, tag="xT")
                if layer + 1 < NLAYER:
                    tstage_n = xtp.tile([128, NPAD], F16, tag="tstage")
                    tables_n = [None, None]
                goffs = {0: 0, 1: 0}      # gather idx cursor per stream
                qsel = 0
                for c in range(NCHUNK):
                    # gather: sub-calls of <= SUBT tiles (descriptor-ring cap)
                    gtiles = {}
                    for s in range(2):
                        tc_s = call_tiles[s][c]
                        view = tables[s][:]
                        subs = []
                        for t0 in range(0, tc_s, SUBT):
                            nt = min(SUBT, tc_s - t0)
                            gb = gbp.tile([128, SUBT, D], F16, tag=f"gb{s}")
                            ni = nt * SLOTS
                            if "gather" not in SKIP:
                                nc.gpsimd.dma_gather(
                                    gb[:, :nt, :], view, gidx_s[s][
                                        :, goffs[s]:goffs[s] + ni // 16],
                                    ni, ni, D, queue_num=qsel % NQ)
                                qsel += 1
                            else:
                                nc.vector.memset(gb[:, :nt, :], 0.0)
                            goffs[s] += ni // 16
                            subs.append(gb)
                        gtiles[s] = subs
                    st = stp.tile([128, CHUNK * R], F16, tag="stile")
                    for b in range(BANKS_PER_CHUNK):
                        bg = c * BANKS_PER_CHUNK + b
                        pb = pbank.tile([128, BANK], F32, space="PSUM",
                                        tag="pbank")
                        n_mm = len(spans[0][bg]) + len(spans[1][bg])
                        i_mm = 0
                        for s in range(2):
                            ms = msbp.tile([128, BANK], F16, tag="msb")
                            nc.sync.dma_start(
                                ms[:], mmat[:, (s * NBANK + bg) * BANK:
                                            (s * NBANK + bg + 1) * BANK])
                            lo = 0
                            tloc = sum(ntiles[s][c * BANKS_PER_CHUNK + bb]
                                       for bb in range(b))
                            for w in spans[s][bg]:
                                nc.tensor.matmul(
                                    pb[:, lo:lo + w],
                                    lhsT=gtiles[s][tloc // SUBT][
                                        :, tloc % SUBT, :],
                                    rhs=ms[:, lo:lo + w],
                                    start=(i_mm == 0),
                                    stop=(i_mm == n_mm - 1))
                                lo += w
                                tloc += 1
                                i_mm += 1
                            assert lo == BANK
                        assert i_mm == n_mm
                        nc.vector.tensor_copy(st[:, b * BANK:(b + 1) * BANK],
                                              pb[:])
                    # phase 2: per-relation + root matmuls
                    cs = slice(c * CHUNK, (c + 1) * CHUNK)
                    po = pbig.tile([128, CHUNK], F32, space="PSUM", tag="pbig")
                    str_ap = st[:].rearrange("p (n r) -> p r n", r=R)
                    for r in range(R):
                        nc.tensor.matmul(po[:],
                                         lhsT=relw_s[:, r * 128:(r + 1) * 128],
                                         rhs=str_ap[:, r, :],
                                         start=(r == 0), stop=False)
                    nc.tensor.matmul(po[:], lhsT=rootw_s[:], rhs=xT[:, cs],
                                     start=False, stop=True)
                    nc.scalar.activation(xTn[:, cs], po[:],
                                         mybir.ActivationFunctionType.Identity,
                                         bias=rgcn_b_s[:], scale=1.0)
                    if layer + 1 < NLAYER:
                        build_tables_postchunk(layer + 1, c, xTn, tstage_n,
                                               tables_n)
                xT = xTn
                if layer + 1 < NLAYER:
                    tstage = tstage_n
                    tables = tables_n

            # ---- output head ----
            for c in range(NCHUNK):
                cs = slice(c * CHUNK, (c + 1) * CHUNK)
                p1 = pbig.tile([128, CHUNK], F32, space="PSUM", tag="pbig")
                nc.tensor.matmul(p1[:], lhsT=wo1_s[:], rhs=xT[:, cs],
                                 start=True, stop=True)
                h = smallp.tile([128, CHUNK], F16, tag="x0")
                nc.scalar.activation(h[:], p1[:],
                                     mybir.ActivationFunctionType.Lrelu,
                                     bias=b_o1_s[:], scale=1.0, alpha=0.01)
                p2 = ptpp.tile([2, CHUNK], F32, space="PSUM", tag="ptp")
                nc.tensor.matmul(p2[:], lhsT=wo2_s[:], rhs=h[:],
                                 start=True, stop=True)
                ot = smallp.tile([2, CHUNK], F32, tag="ot")
                nc.scalar.activation(ot[:], p2[:],
                                     mybir.ActivationFunctionType.Identity,
                                     bias=b_o2_s[:], scale=1.0)
                nc.sync.dma_start(outT[:, cs], ot[:])

    nc.compile()
    return nc


# ---------------- host wrapper ----------------
def _pack_inputs(inputs, plan):
    f16 = np.float16
    des, tweet = inputs["des"], inputs["tweet"]
    num_prop, cat_prop = inputs["num_prop"], inputs["cat_prop"]
    order = plan["order"]

    w_blk = np.zeros((FINP, 128), np.float32)
    w_blk[0:768, 0:32] = inputs["W_des"]
    w_blk[768:1536, 32:64] = inputs["W_tw"]
    w_blk[1536:1542, 64:96] = inputs["W_np"]
    w_blk[1542:1553, 96:128] = inputs["W_cp"]
    w_all = np.concatenate([w_blk[f * 128:(f + 1) * 128, :]
                            for f in range(13)], axis=1).astype(f16)
    b_x0 = np.concatenate([inputs["b_des"], inputs["b_tw"],
                           inputs["b_np"], inputs["b_cp"]]
                          ).astype(np.float32).reshape(128, 1)
    relw = np.concatenate([inputs["rel_w"][r] for r in range(R)],
                          axis=1).astype(f16)

    in_maps = []
    for k in range(NCORES):
        rows = k * NLOC + order[k]                   # permuted global rows
        feat = np.zeros((FINP, NPAD), f16)
        feat[0:768, :NLOC] = des[rows].T
        feat[768:1536, :NLOC] = tweet[rows].T
        feat[1536:1542, :NLOC] = num_prop[rows].T
        feat[1542:1553, :NLOC] = cat_prop[rows].T
        m = {
            "featT": feat,
            "w_all": w_all,
            "b_x0": b_x0,
            "w_in": inputs["W_in"].astype(f16),
            "b_in": inputs["b_in"].astype(np.float32).reshape(128, 1),
            "relw": relw,
            "rootw": inputs["root_w"].astype(f16),
            "rgcn_b": inputs["rgcn_b"].astype(np.float32).reshape(128, 1),
            "wo1": inputs["W_o1"].astype(f16),
            "b_o1": inputs["b_o1"].astype(np.float32).reshape(128, 1),
            "wo2": inputs["W_o2"].astype(f16),
            "b_o2": inputs["b_o2"].astype(np.float32).reshape(2, 1),
            "gidxA": plan["gidx"][0][k],
            "gidxB": plan["gidx"][1][k],
            "mmat": plan["mmat"][k],
        }
        in_maps.append(m)
    return in_maps


def _get_compiled(edge_index, edge_type):
    key = hash((np.asarray(edge_index).tobytes(),
                np.asarray(edge_type).tobytes()))
    if key not in _CACHE:
        t0 = time.time()
        plan = _plan_graph(edge_index, edge_type)
        t1 = time.time()
        nc = _build_nc(plan)
        t2 = time.time()
        print(f"[kernel] plan {t1-t0:.0f}s, build+compile {t2-t1:.0f}s",
              flush=True)
        _CACHE[key] = (nc, plan)
    return _CACHE[key]


def kernel(trace=False, **inputs):
    nc, plan = _get_compiled(inputs["edge_index"], inputs["edge_type"])
    in_maps = _pack_inputs(inputs, plan)
    t0 = time.time()
    res = run_bass_kernel_spmd(nc, in_maps, list(range(NCORES)), trace=trace)
    print(f"[kernel] run {time.time()-t0:.0f}s", flush=True)
    out = np.zeros((N, 2), np.float32)
    order = plan["order"]
    for k in range(NCORES):
        out[k * NLOC + order[k]] = res.results[k]["outT"][:, :NLOC].T
    if trace:
        return out, res
    return out


# revision 6
# speedup vs baseline: 1.3112x; 1.0476x over previous
"""BotRGCN (4 shared RGCN layers) on 8 TRN2 NeuronCores via Bass/Tile.

Strategy (sharding_hint): nodes sharded across 8 cores (6250 each, padded to
6656 = 13*512); edges partitioned by destination core and sorted by
(dst_local, rel) segment; per layer an AllGather replicates the row-major x
table (fp16) to every core's DRAM, then each core dma_gathers its edges'
source rows and computes segment means via PE matmuls against 0/1 membership
matrices (fp8, resident in SBUF; the per-segment 1/cnt is applied as an
exact per-slot scale on the gathered tiles). Per-relation RGCN weights +
root term are dense PE matmuls; small weights replicated.

v2 over baseline:
  - per-core degree-descending node permutation aligns edge counts across
    cores, tightening the shared span packing (fewer gather slots/tiles);
  - membership matrices are 0/1 fp8 and SBUF-resident (loaded once) instead
    of 17MB/layer of fp16 streamed from HBM; 1/cnt moves to a DVE
    per-partition scale of the gathered tiles (exact);
  - next layer's row-major table is built chunk-by-chunk during the current
    layer and each AllGather half fires as soon as its rows are ready.

Self-contained: hardcodes all shapes from the problem spec.
"""
import os
import time

import numpy as np
import ml_dtypes

import concourse.bacc as bacc
import concourse.bass as bass
import concourse.mybir as mybir
import concourse.tile as tile
from concourse.bass_utils import run_bass_kernel_spmd
from concourse.masks import make_identity

# ---------------- problem constants (hardcoded from spec) ----------------
NCORES = 8
N = 50000
E = 800000
R = 5
D = 128
FIN = 768 + 768 + 6 + 11          # 1553 concat input features
FINP = 13 * 128                   # padded to 1664
NLOC = N // NCORES                # 6250
CHUNK = 512                       # nodes per chunk
NCHUNK = 13
NPAD = NCHUNK * CHUNK             # 6656 padded nodes/core
NTAB = NCORES * NPAD              # 53248 table rows
BANK = 512                        # segment columns per PSUM bank
BANKS_PER_CHUNK = CHUNK * R // BANK   # 5
NBANK = NCHUNK * BANKS_PER_CHUNK  # 65
NSEG = NPAD * R                   # 33280 dense segment grid per core
HALFROW = NPAD // 2               # 3328: rows per half-table shard
NTABH = NCORES * HALFROW          # 26624 rows per half table (< 32768)
SLOTS = 128                       # edge slots per tile
SUBT = int(os.environ.get("KB_SUBT", "8"))   # gather tiles per call
NQ = int(os.environ.get("KB_NQ", "4"))        # SWDGE queues (cpu-pair select)
SCRATCH = int(os.environ.get("KB_SCRATCH", "65536"))
GMODE = os.environ.get("KB_GATHER", "dge")      # dge | indirect
GBUF = int(os.environ.get("KB_GBUF", "8"))      # gather tile pool depth
NLAYER = int(os.environ.get("KB_LAYERS", "4"))
SKIP = set(os.environ.get("KB_SKIP", "").split(","))

F16 = mybir.dt.float16
F32 = mybir.dt.float32
F8 = mybir.dt.float8e4
I16 = mybir.dt.int16

_CACHE = {}


# ---------------- host-side graph preprocessing ----------------
def _plan_graph(edge_index, edge_type):
    """Build per-core tile structure. Span layout is shared by all cores
    (SPMD: one program), per-core data (idx, M, inv) differs.

    Nodes within each core are permuted by descending total degree so the
    per-column edge counts correlate across cores — the greedy shared span
    packing then wastes fewer slots on the max-core constraint."""
    src = np.asarray(edge_index[0], dtype=np.int64)
    dst = np.asarray(edge_index[1], dtype=np.int64)
    et = np.asarray(edge_type, dtype=np.int64)

    # per-core degree-descending permutation: order[k][j] = original local
    # id of the node placed at permuted position j on core k.
    dst_core_all = dst // NLOC
    dst_loc_all = dst % NLOC
    order = np.zeros((NCORES, NLOC), dtype=np.int64)
    pos_of = np.zeros((NCORES, NLOC), dtype=np.int64)
    for k in range(NCORES):
        degk = np.bincount(dst_loc_all[dst_core_all == k], minlength=NLOC)
        o = np.argsort(-degk, kind="stable")
        order[k] = o
        pos_of[k, o] = np.arange(NLOC)

    core = dst_core_all
    col = pos_of[core, dst_loc_all] * R + et          # permuted (node, rel)
    src_core = src // NLOC
    src_pos = pos_of[src_core, src % NLOC]            # permuted src position
    stream = (src_pos >= HALFROW).astype(np.int64)    # src half
    src_pad = src_core * HALFROW + (src_pos - stream * HALFROW)

    # per (core, stream): edges sorted by col
    edges = {}
    counts = np.zeros((NCORES, 2, NSEG), dtype=np.int64)
    for k in range(NCORES):
        for s in range(2):
            m = (core == k) & (stream == s)
            c = col[m]
            o = np.argsort(c, kind="stable")
            edges[(k, s)] = (c[o], src_pad[m][o])
            np.add.at(counts[k, s], c[o], 1)

    cnt_total = counts.sum(axis=1)                    # [NCORES, NSEG]
    invc = 1.0 / np.maximum(cnt_total, 1.0)           # per core

    # static spans per (stream, bank): greedy, max-over-cores count <= SLOTS
    spans = {0: [], 1: []}                            # spans[s][b] = [widths]
    for s in range(2):
        for b in range(NBANK):
            base = b * BANK
            cc = counts[:, s, base:base + BANK]       # [NCORES, BANK]
            assert cc.max(initial=0) <= SLOTS, "single segment exceeds tile"
            widths = []
            run = np.zeros(NCORES, dtype=np.int64)
            w = 0
            for j in range(BANK):
                if (run + cc[:, j]).max() > SLOTS:
                    widths.append(w)
                    run[:] = 0
                    w = 0
                run += cc[:, j]
                w += 1
            widths.append(w)
            spans[s].append(widths)

    ntiles = {s: [len(spans[s][b]) for b in range(NBANK)] for s in range(2)}
    call_tiles = {s: [sum(ntiles[s][c * BANKS_PER_CHUNK + b]
                          for b in range(BANKS_PER_CHUNK))
                      for c in range(NCHUNK)] for s in range(2)}
    tot_tiles = {s: sum(ntiles[s]) for s in range(2)}
    tt_all = tot_tiles[0] + tot_tiles[1]

    # per-core data: gather idx (wrapped int16), 0/1 M matrices (fp8),
    # per-slot 1/cnt scales
    gidx = {s: np.zeros((NCORES, 128, tot_tiles[s] * SLOTS // 16), np.int16)
            for s in range(2)}
    goff = {s: np.zeros((NCORES, 128, tot_tiles[s]), np.int32)
            for s in range(2)}
    mmat = np.zeros((NCORES, 128, 2 * NBANK * BANK), np.float16)
    for k in range(NCORES):
        for s in range(2):
            cols_e, srcp_e = edges[(k, s)]
            flat_idx = np.zeros(tot_tiles[s] * SLOTS, np.int16)
            tglob = 0
            for b in range(NBANK):
                base = b * BANK
                lo = 0
                for w in spans[s][b]:
                    e0 = np.searchsorted(cols_e, base + lo)
                    e1 = np.searchsorted(cols_e, base + lo + w)
                    nslot = e1 - e0
                    assert nslot <= SLOTS
                    flat_idx[tglob * SLOTS:tglob * SLOTS + nslot] = \
                        srcp_e[e0:e1]
                    mcol = (s * NBANK + b) * BANK + (cols_e[e0:e1] - base)
                    mmat[k, np.arange(nslot), mcol] = \
                        invc[k][cols_e[e0:e1]].astype(np.float16)
                    lo += w
                    tglob += 1
            # wrap: element i -> [i%16, i//16], replicated across 8 groups
            wr = flat_idx.reshape(-1, 16).T            # [16, ntot*8]
            gidx[s][k] = np.tile(wr, (8, 1))
            # per-partition layout for indirect mode: off[p, t] = idx[t*128+p]
            goff[s][k] = flat_idx.reshape(-1, 128).T.astype(np.int32)
    return dict(spans=spans, ntiles=ntiles, call_tiles=call_tiles,
                tot_tiles=tot_tiles, gidx=gidx, goff=goff, mmat=mmat,
                order=order)


# ---------------- device program ----------------
def _build_nc(plan):
    nc = bacc.Bacc("TRN2", target_bir_lowering=False, debug=False,
                   num_devices=NCORES, num_swdge_queues=NQ,
                   dynamic_dma_scratch_size=SCRATCH)
    spans, ntiles = plan["spans"], plan["ntiles"]
    call_tiles, tot_tiles = plan["call_tiles"], plan["tot_tiles"]
    tt_all = tot_tiles[0] + tot_tiles[1]

    # inputs (per core)
    featT = nc.dram_tensor("featT", [FINP, NPAD], F16, kind="ExternalInput")
    w_all = nc.dram_tensor("w_all", [128, 13 * 128], F16, kind="ExternalInput")
    b_x0 = nc.dram_tensor("b_x0", [128, 1], F32, kind="ExternalInput")
    w_in = nc.dram_tensor("w_in", [128, 128], F16, kind="ExternalInput")
    b_in = nc.dram_tensor("b_in", [128, 1], F32, kind="ExternalInput")
    relw = nc.dram_tensor("relw", [128, R * 128], F16, kind="ExternalInput")
    rootw = nc.dram_tensor("rootw", [128, 128], F16, kind="ExternalInput")
    rgcn_b = nc.dram_tensor("rgcn_b", [128, 1], F32, kind="ExternalInput")
    wo1 = nc.dram_tensor("wo1", [128, 128], F16, kind="ExternalInput")
    b_o1 = nc.dram_tensor("b_o1", [128, 1], F32, kind="ExternalInput")
    wo2 = nc.dram_tensor("wo2", [128, 2], F16, kind="ExternalInput")
    b_o2 = nc.dram_tensor("b_o2", [2, 1], F32, kind="ExternalInput")
    gidxA = nc.dram_tensor("gidxA", [128, tot_tiles[0] * 8], I16,
                           kind="ExternalInput")
    gidxB = nc.dram_tensor("gidxB", [128, tot_tiles[1] * 8], I16,
                           kind="ExternalInput")
    mmat = nc.dram_tensor("mmat", [128, 2 * NBANK * BANK], F16,
                          kind="ExternalInput")
    goffA = nc.dram_tensor("goffA", [128, tot_tiles[0]], mybir.dt.int32,
                           kind="ExternalInput")
    goffB = nc.dram_tensor("goffB", [128, tot_tiles[1]], mybir.dt.int32,
                           kind="ExternalInput")
    outT = nc.dram_tensor("outT", [2, NPAD], F32, kind="ExternalOutput")

    with tile.TileContext(nc) as tc:
        with (
            tc.tile_pool(name="const", bufs=1) as constp,
            tc.tile_pool(name="xt", bufs=2) as xtp,
            tc.tile_pool(name="feat", bufs=3) as featp,
            tc.tile_pool(name="gb", bufs=GBUF) as gbp,
            tc.tile_pool(name="msb", bufs=3) as msbp,
            tc.tile_pool(name="stile", bufs=2) as stp,
            tc.tile_pool(name="small", bufs=3) as smallp,
            tc.tile_pool(name="pbank", bufs=3, space="PSUM") as pbank,
            tc.tile_pool(name="pbig", bufs=2, space="PSUM") as pbig,
            tc.tile_pool(name="ptp", bufs=2, space="PSUM") as ptpp,
            tc.tile_pool(name="dram", bufs=1, space="DRAM") as dramp,
            tc.tile_pool(name="shared", bufs=1, space="DRAM") as sharedp,
        ):
            # ---- resident constants ----
            def load_const(t, shape, dt):
                s = constp.tile(shape, dt, tag=t.name)
                nc.sync.dma_start(s[:], t[:])
                return s
            w_all_s = load_const(w_all, [128, 13 * 128], F16)
            b_x0_s = load_const(b_x0, [128, 1], F32)
            w_in_s = load_const(w_in, [128, 128], F16)
            b_in_s = load_const(b_in, [128, 1], F32)
            relw_s = load_const(relw, [128, R * 128], F16)
            rootw_s = load_const(rootw, [128, 128], F16)
            rgcn_b_s = load_const(rgcn_b, [128, 1], F32)
            wo1_s = load_const(wo1, [128, 128], F16)
            b_o1_s = load_const(b_o1, [128, 1], F32)
            wo2_s = load_const(wo2, [128, 2], F16)
            b_o2_s = load_const(b_o2, [2, 1], F32)
            gidx_s = [load_const(gidxA, [128, tot_tiles[0] * 8], I16),
                      load_const(gidxB, [128, tot_tiles[1] * 8], I16)]
            goff_s = [load_const(goffA, [128, tot_tiles[0]], mybir.dt.int32),
                      load_const(goffB, [128, tot_tiles[1]], mybir.dt.int32)]
            ident = constp.tile([128, 128], F16, tag="ident")
            make_identity(nc, ident[:])

            # table build: transpose the 4 blocks of chunk c of `xsrc` into
            # `tstage`; on the half boundaries ship + AllGather into `tables`
            def build_tables_postchunk(layer, c, xsrc, tstage, tables):
                for j in range(4 * c, 4 * c + 4):
                    pt = ptpp.tile([128, 128], F16, space="PSUM", tag="ptp")
                    nc.tensor.transpose(pt[:], xsrc[:, j * 128:(j + 1) * 128],
                                        ident[:])
                    nc.vector.tensor_copy(tstage[:, j * 128:(j + 1) * 128],
                                          pt[:])
                for h in range(2):
                    if c != (6 if h == 0 else 12):
                        continue
                    tb = sharedp.tile([NTABH, D], F16, addr_space="Shared",
                                      tag=f"table{layer}_{h}")
                    tsh = dramp.tile([HALFROW, D], F16, tag=f"tsh{layer}_{h}")
                    nc.sync.dma_start(
                        tsh[:].rearrange("(j p) d -> p j d", p=128),
                        tstage[:, h * HALFROW:(h + 1) * HALFROW].rearrange(
                            "p (j d) -> p j d", d=D))
                    if "coll" not in SKIP:
                        nc.gpsimd.collective_compute(
                            "AllGather", mybir.AluOpType.bypass,
                            replica_groups=[list(range(NCORES))],
                            ins=[tsh[:].opt()], outs=[tb[:].opt()])
                    else:
                        nc.sync.dma_start(tb[NPAD // 2:NPAD, :], tsh[:])
                    tables[h] = tb

            # ---- input projection -> xT [128, NPAD] fp16 (+ layer-0 table)
            xT = xtp.tile([128, NPAD], F16, tag="xT")
            tstage = xtp.tile([128, NPAD], F16, tag="tstage")
            tables = [None, None]
            for c in range(NCHUNK):
                cs = slice(c * CHUNK, (c + 1) * CHUNK)
                p0 = pbig.tile([128, CHUNK], F32, space="PSUM", tag="pbig")
                for f in range(13):
                    ft = featp.tile([128, CHUNK], F16, tag="feat")
                    nc.sync.dma_start(ft[:], featT[f * 128:(f + 1) * 128, cs])
                    nc.tensor.matmul(p0[:],
                                     lhsT=w_all_s[:, f * 128:(f + 1) * 128],
                                     rhs=ft[:], start=(f == 0), stop=(f == 12))
                x0 = smallp.tile([128, CHUNK], F16, tag="x0")
                nc.scalar.activation(x0[:], p0[:],
                                     mybir.ActivationFunctionType.Lrelu,
                                     bias=b_x0_s[:], scale=1.0, alpha=0.01)
                p1 = pbig.tile([128, CHUNK], F32, space="PSUM", tag="pbig")
                nc.tensor.matmul(p1[:], lhsT=w_in_s[:], rhs=x0[:],
                                 start=True, stop=True)
                nc.scalar.activation(xT[:, cs], p1[:],
                                     mybir.ActivationFunctionType.Lrelu,
                                     bias=b_in_s[:], scale=1.0, alpha=0.01)
                build_tables_postchunk(0, c, xT, tstage, tables)

            # ---- RGCN layers ----
            for layer in range(NLAYER):
                xTn = xtp.tile([128, NPAD], F16, tag="xT")
                if layer + 1 < NLAYER:
                    tstage_n = xtp.tile([128, NPAD], F16, tag="tstage")
                    tables_n = [None, None]
                goffs = {0: 0, 1: 0}      # gather idx cursor per stream
                qsel = 0
                for c in range(NCHUNK):
                    # gather: sub-calls of <= SUBT tiles (descriptor-ring cap)
                    gtiles = {}
                    for s in range(2):
                        tc_s = call_tiles[s][c]
                        view = tables[s][:]
                        subs = []
                        for t0 in range(0, tc_s, SUBT):
                            nt = min(SUBT, tc_s - t0)
                            gb = gbp.tile([128, SUBT, D], F16, tag=f"gb{s}")
                            ni = nt * SLOTS
                            if "gather" in SKIP:
                                nc.vector.memset(gb[:, :nt, :], 0.0)
                            elif GMODE == "indirect":
                                tb0 = goffs[s] // 8
                                nc.gpsimd.indirect_dma_start(
                                    out=gb[:, :nt, :],
                                    out_offset=None,
                                    in_=view,
                                    in_offset=bass.IndirectOffsetOnAxis(
                                        ap=goff_s[s][:, tb0:tb0 + nt],
                                        axis=0,
                                    ),
                                )
                            else:
                                nc.gpsimd.dma_gather(
                                    gb[:, :nt, :], view, gidx_s[s][
                                        :, goffs[s]:goffs[s] + ni // 16],
                                    ni, ni, D, queue_num=qsel % NQ)
                                qsel += 1
                            goffs[s] += ni // 16
                            subs.append(gb)
                        gtiles[s] = subs
                    st = stp.tile([128, CHUNK * R], F16, tag="stile")
                    for b in range(BANKS_PER_CHUNK):
                        bg = c * BANKS_PER_CHUNK + b
                        pb = pbank.tile([128, BANK], F32, space="PSUM",
                                        tag="pbank")
                        n_mm = len(spans[0][bg]) + len(spans[1][bg])
                        i_mm = 0
                        for s in range(2):
                            ms = msbp.tile([128, BANK], F16, tag="msb")
                            nc.sync.dma_start(
                                ms[:], mmat[:, (s * NBANK + bg) * BANK:
                                            (s * NBANK + bg + 1) * BANK])
                            lo = 0
                            tloc = sum(ntiles[s][c * BANKS_PER_CHUNK + bb]
                                       for bb in range(b))
                            for w in spans[s][bg]:
                                nc.tensor.matmul(
                                    pb[:, lo:lo + w],
                                    lhsT=gtiles[s][tloc // SUBT][
                                        :, tloc % SUBT, :],
                                    rhs=ms[:, lo:lo + w],
                                    start=(i_mm == 0),
                                    stop=(i_mm == n_mm - 1))
                                lo += w
                                tloc += 1
                                i_mm += 1
                            assert lo == BANK
                        assert i_mm == n_mm
                        nc.vector.tensor_copy(st[:, b * BANK:(b + 1) * BANK],
                                              pb[:])
                    # phase 2: per-relation + root matmuls
                    cs = slice(c * CHUNK, (c + 1) * CHUNK)
                    po = pbig.tile([128, CHUNK], F32, space="PSUM", tag="pbig")
                    str_ap = st[:].rearrange("p (n r) -> p r n", r=R)
                    for r in range(R):
                        nc.tensor.matmul(po[:],
                                         lhsT=relw_s[:, r * 128:(r + 1) * 128],
                                         rhs=str_ap[:, r, :],
                                         start=(r == 0), stop=False)
                    nc.tensor.matmul(po[:], lhsT=rootw_s[:], rhs=xT[:, cs],
                                     start=False, stop=True)
                    nc.scalar.activation(xTn[:, cs], po[:],
                                         mybir.ActivationFunctionType.Identity,
                                         bias=rgcn_b_s[:], scale=1.0)
                    if layer + 1 < NLAYER:
                        build_tables_postchunk(layer + 1, c, xTn, tstage_n,
                                               tables_n)
                xT = xTn
                if layer + 1 < NLAYER:
                    tstage = tstage_n
                    tables = tables_n

            # ---- output head ----
            for c in range(NCHUNK):
                cs = slice(c * CHUNK, (c + 1) * CHUNK)
                p1 = pbig.tile([128, CHUNK], F32, space="PSUM", tag="pbig")
                nc.tensor.matmul(p1[:], lhsT=wo1_s[:], rhs=xT[:, cs],
                                 start=True, stop=True)
                h = smallp.tile([128, CHUNK], F16, tag="x0")
                nc.scalar.activation(h[:], p1[:],
                                     mybir.ActivationFunctionType.Lrelu,
                                     bias=b_o1_s[:], scale=1.0, alpha=0.01)
                p2 = ptpp.tile([2, CHUNK], F32, space="PSUM", tag="ptp")
                nc.tensor.matmul(p2[:], lhsT=wo2_s[:], rhs=h[:],
                                 start=True, stop=True)
                ot = smallp.tile([2, CHUNK], F32, tag="ot")
                nc.scalar.activation(ot[:], p2[:],
                                     mybir.ActivationFunctionType.Identity,
                                     bias=b_o2_s[:], scale=1.0)
                nc.sync.dma_start(outT[:, cs], ot[:])

    nc.compile()
    return nc


# ---------------- host wrapper ----------------
def _pack_inputs(inputs, plan):
    f16 = np.float16
    des, tweet = inputs["des"], inputs["tweet"]
    num_prop, cat_prop = inputs["num_prop"], inputs["cat_prop"]
    order = plan["order"]

    w_blk = np.zeros((FINP, 128), np.float32)
    w_blk[0:768, 0:32] = inputs["W_des"]
    w_blk[768:1536, 32:64] = inputs["W_tw"]
    w_blk[1536:1542, 64:96] = inputs["W_np"]
    w_blk[1542:1553, 96:128] = inputs["W_cp"]
    w_all = np.concatenate([w_blk[f * 128:(f + 1) * 128, :]
                            for f in range(13)], axis=1).astype(f16)
    b_x0 = np.concatenate([inputs["b_des"], inputs["b_tw"],
                           inputs["b_np"], inputs["b_cp"]]
                          ).astype(np.float32).reshape(128, 1)
    relw = np.concatenate([inputs["rel_w"][r] for r in range(R)],
                          axis=1).astype(f16)

    in_maps = []
    for k in range(NCORES):
        rows = k * NLOC + order[k]                   # permuted global rows
        feat = np.zeros((FINP, NPAD), f16)
        feat[0:768, :NLOC] = des[rows].T
        feat[768:1536, :NLOC] = tweet[rows].T
        feat[1536:1542, :NLOC] = num_prop[rows].T
        feat[1542:1553, :NLOC] = cat_prop[rows].T
        m = {
            "featT": feat,
            "w_all": w_all,
            "b_x0": b_x0,
            "w_in": inputs["W_in"].astype(f16),
            "b_in": inputs["b_in"].astype(np.float32).reshape(128, 1),
            "relw": relw,
            "rootw": inputs["root_w"].astype(f16),
            "rgcn_b": inputs["rgcn_b"].astype(np.float32).reshape(128, 1),
            "wo1": inputs["W_o1"].astype(f16),
            "b_o1": inputs["b_o1"].astype(np.float32).reshape(128, 1),
            "wo2": inputs["W_o2"].astype(f16),
            "b_o2": inputs["b_o2"].astype(np.float32).reshape(2, 1),
            "gidxA": plan["gidx"][0][k],
            "gidxB": plan["gidx"][1][k],
            "goffA": plan["goff"][0][k],
            "goffB": plan["goff"][1][k],
            "mmat": plan["mmat"][k],
        }
        in_maps.append(m)
    return in_maps


def _get_compiled(edge_index, edge_type):
    key = hash((np.asarray(edge_index).tobytes(),
                np.asarray(edge_type).tobytes()))
    if key not in _CACHE:
        t0 = time.time()
        plan = _plan_graph(edge_index, edge_type)
        t1 = time.time()
        nc = _build_nc(plan)
        t2 = time.time()
        print(f"[kernel] plan {t1-t0:.0f}s, build+compile {t2-t1:.0f}s",
              flush=True)
        _CACHE[key] = (nc, plan)
    return _CACHE[key]


def kernel(trace=False, **inputs):
    nc, plan = _get_compiled(inputs["edge_index"], inputs["edge_type"])
    in_maps = _pack_inputs(inputs, plan)
    t0 = time.time()
    res = run_bass_kernel_spmd(nc, in_maps, list(range(NCORES)), trace=trace)
    print(f"[kernel] run {time.time()-t0:.0f}s", flush=True)
    out = np.zeros((N, 2), np.float32)
    order = plan["order"]
    for k in range(NCORES):
        out[k * NLOC + order[k]] = res.results[k]["outT"][:, :NLOC].T
    if trace:
        return out, res
    return out
